# revision 4
# baseline (speedup 1.0000x reference)
"""KoLeo loss kernel for Trainium2, distributed over 8 NeuronCores.

Math: with xn = row-normalized x, the loss is
    loss = -mean_i log(||xn_i - xn_{nn(i)}|| + eps),  nn(i) = argmax_{j!=i} cos(i,j)
Since ||xn_i - xn_j||^2 = 2 - 2*cos(i,j), only the row max of the cosine
similarity matrix is needed:
    loss = -0.5 * mean_i log(2 - 2*max_{j!=i} cos(i,j))
(The eps terms contribute ~1e-8 relative — far below fp32 noise.)

Sharding: core c gets xk = roll(x, -c*2048, rows), so its 2048 query rows sit
at rows 0..2047 of its own input and the diagonal (self-match) block lands at
a compile-time-known column block, keeping the NEFF SPMD-uniform. Each core:
  1. normalizes all 16384 rows (ACT square+accum -> sqrt -> DVE reciprocal ->
     ACT scale-copy to fp16),
  2. transposes to feature-major via xbar DMA transpose into SBUF,
  3. computes its 2048x16384 similarity slice on the PE in fp16 (PSUM fp32),
     subtracting 3 on the diagonal via one extra identity-matmul pass,
  4. max-reduces each PSUM group on DVE, then log + partial sum on ACT.
Host sums the 8 per-core [128,1] partials: loss = -0.5 * total / N.
"""

import numpy as np

N = 16384
D = 768
NCORES = 8
SLICE = N // NCORES  # 2048 query rows per core

P = 128           # partitions
KSUB = D // P     # 6 contraction sub-tiles
NCHUNK = 512      # key columns per chunk tile
NUM_CHUNKS = N // NCHUNK          # 32
BANKS = 4                         # psum banks per group
NGRP = NUM_CHUNKS // BANKS        # 8 column groups of 2048
MT = SLICE // P                   # 16 query row chunks
ROWTILES = N // P                 # 128

_cached_nc = None


def _build_module():
    import concourse.bacc as bacc
    import concourse.mybir as mybir
    import concourse.tile as tile
    from concourse.masks import make_identity

    f32 = mybir.dt.float32
    f16 = mybir.dt.float16
    AF = mybir.ActivationFunctionType

    nc = bacc.Bacc("TRN2", target_bir_lowering=False, debug=False)
    x = nc.dram_tensor("xk", [N, D], f32, kind="ExternalInput").ap()
    out = nc.dram_tensor("out", [P, 1], f32, kind="ExternalOutput").ap()

    with tile.TileContext(nc) as tc:
        consts = tc.alloc_tile_pool(name="consts", bufs=1)
        q_pool = tc.alloc_tile_pool(name="qchunks", bufs=4)
        ring = tc.alloc_tile_pool(name="ring", bufs=8)
        xf_pool = tc.alloc_tile_pool(name="xf", bufs=3)
        sq_pool = tc.alloc_tile_pool(name="sq", bufs=2)
        xh_pool = tc.alloc_tile_pool(name="xh", bufs=3)
        s_pool = tc.alloc_tile_pool(name="stats", bufs=8)
        misc = tc.alloc_tile_pool(name="misc", bufs=1)
        psum_pool = tc.alloc_tile_pool(name="psum", bufs=2, space="PSUM")

        # constants: identity (for the diagonal-mask matmul) and a shifted
        # -3*I block so every mask offset is a static slice of one tensor
        ident = consts.tile([P, P], f16)
        make_identity(nc, ident)
        m3 = consts.tile([P, 896], f16)
        nc.gpsimd.memset(m3, 0.0)
        # fill -3 where (384 + p - y) == 0, i.e. y = p + 384
        nc.gpsimd.affine_select(
            out=m3, in_=m3, compare_op=mybir.AluOpType.not_equal,
            fill=-3.0, base=384, pattern=[[-1, 896]], channel_multiplier=1,
        )

        # ---- phase 1: normalize + transpose to feature-major fp16 chunks ----
        chunks = [None] * NUM_CHUNKS
        for t in range(ROWTILES):
            r0 = t * P
            xf = xf_pool.tile([P, D], f32, tag="xf")
            nc.sync.dma_start(xf, x[r0:r0 + P, :])
            sq = sq_pool.tile([P, D], f16, tag="sq")
            ssq = s_pool.tile([P, 1], f32, tag="ssq")
            nc.scalar.activation(out=sq, in_=xf, func=AF.Square, accum_out=ssq)
            nrm = s_pool.tile([P, 1], f32, tag="nrm")
            nc.scalar.activation(out=nrm, in_=ssq, func=AF.Sqrt)
            rs = s_pool.tile([P, 1], f32, tag="rs")
            nc.vector.reciprocal(out=rs, in_=nrm)
            xh = xh_pool.tile([P, D], f16, tag="xh")
            nc.scalar.activation(out=xh, in_=xf, func=AF.Copy, scale=rs)

            c = t // 4
            off = (t % 4) * P
            if t % 4 == 0:
                pool = q_pool if c < BANKS else ring
                chunks[c] = pool.tile(
                    [P, KSUB, NCHUNK], f16, tag="chunk", name=f"chunk{c}"
                )
            for k in range(KSUB):
                nc.sync.dma_start_transpose(
                    out=chunks[c][:, k, off:off + P],
                    in_=xh[:, k * P:(k + 1) * P],
                )

        # ---- phase 2: similarity matmuls + running row-max ----
        colmax = misc.tile([P, MT, NGRP], f32)
        for ng in range(NGRP):
            for m in range(MT):
                mc, mo = m // 4, (m % 4) * P
                ps = psum_pool.tile([P, BANKS, NCHUNK], f32, tag="ps")
                for b in range(BANKS):
                    j = ng * BANKS + b
                    masked = (ng == 0 and b == mc)
                    for k in range(KSUB):
                        nc.tensor.matmul(
                            ps[:, b, :],
                            lhsT=chunks[mc][:, k, mo:mo + P],
                            rhs=chunks[j][:, k, :],
                            start=(k == 0),
                            stop=(k == KSUB - 1 and not masked),
                        )
                    if masked:
                        s = 384 - mo
                        nc.tensor.matmul(
                            ps[:, b, :], lhsT=ident, rhs=m3[:, s:s + NCHUNK],
                            start=False, stop=True,
                        )
                nc.vector.reduce_max(
                    out=colmax[:, m, ng:ng + 1], in_=ps[:, :, :],
                    axis=mybir.AxisListType.XY,
                )

        # ---- phase 3: loss pieces ----
        mx = misc.tile([P, MT], f32)
        nc.vector.reduce_max(out=mx, in_=colmax, axis=mybir.AxisListType.X)
        lnout = misc.tile([P, MT], f32)
        lsum = misc.tile([P, 1], f32)
        two = misc.tile([P, 1], f32)
        nc.vector.memset(two, 2.0)
        # ln(2 - 2*max) = 2*ln(dist); accum_out sums over the 16 m-chunks
        nc.scalar.activation(
            out=lnout, in_=mx, func=AF.Ln, bias=two, scale=-2.0, accum_out=lsum,
        )
        nc.sync.dma_start(out, lsum)

        for p in (psum_pool, misc, s_pool, xh_pool, sq_pool, xf_pool, ring,
                  q_pool, consts):
            p.release()

    nc.compile()
    return nc


def get_module():
    global _cached_nc
    if _cached_nc is None:
        _cached_nc = _build_module()
    return _cached_nc


def run_cores(x, trace=False, **kw):
    from concourse.bass_utils import run_bass_kernel_spmd

    nc = get_module()
    in_maps = [
        {"xk": np.ascontiguousarray(np.roll(x, -c * SLICE, axis=0))}
        for c in range(NCORES)
    ]
    return run_bass_kernel_spmd(
        nc, in_maps, core_ids=list(range(NCORES)), trace=trace, **kw
    )


def kernel(x):
    x = np.ascontiguousarray(np.asarray(x, dtype=np.float32))
    assert x.shape == (N, D), x.shape
    res = run_cores(x)
    total = sum(
        float(r["out"].astype(np.float64).sum()) for r in res.results
    )
    loss = -0.5 * total / N
    return np.float32(loss)
